# revision 12
# baseline (speedup 1.0000x reference)
"""Trainium2 Bass kernel for nn_Attention_90125593739547.

Full-input contract: kernel(**inputs) takes the unsharded numpy inputs and
returns the full [S, B, D] output. Internally:
  - 8 NeuronCores, core c handles batch b = c // 4 and 4 heads (c % 4).
  - Softmax algebra moves biases off the TensorE: the k-bias shifts all
    logits of a softmax row equally (dropped), the v-bias and output bias
    are linear post-terms (added on host), only the q-bias survives (one
    per-partition DVE add at evacuation).

Scheduling (v2): the kernel is ScalarE-bound (128 exp ACTIVATEs of
(1024+352)/1.2 ns = 147us); the PE must stay >90% busy so the HAM clock
gate never drops it to 1.2 GHz.  The For_i body is unrolled 2x with
double-buffered K/Q/V sets: each body runs pure attention on the current
set while producing the NEXT iteration's K/Q/V + V2 tiles as evenly
spread PE filler.  A prologue outside the loop produces iteration 0's
set; each body's trailing projection (qc3) is carried into the next
body's first stripe; a flush after the loop emits the last one.

Per-core program (bf16 matmuls, fp8e4m3 DoubleRow attn@V):
  kT/qT = W.T @ x          [128 (2 heads x 64), 2048] bf16, N=2048 chains
  V2    = x @ Wv stored per t-tile-pair in a DoubleRow-folded layout
          [128, 4h * 2j * 68]; column 64 of each 68-block is memset to 1
          so the PV matmul also accumulates the softmax denominator.
  per head-pair (m), per q-chunk of 512:
    sc  = kT_h.T @ qT_h per t-pair          [128, 2 * 512] PSUM
    pT  = exp(SCALE * sc + C)               one ScalarE op; C keeps the
                                            fp8 values in normal range
                                            and cancels in the ratio
    pv += V2_pair.T @ pT  (DoubleRow K=256) [65, 512]; row 64 = sum p
    OT  = pv[0:64] * recip(pv[64])
  y_partial = OT.T @ Wp                     [2048, 1024] bf16 out
Host sums the 4 per-head-group partials per batch and adds bv@Wp + bp.
"""
import sys
sys.path.insert(0, '/opt/trn_rl_repo')
import numpy as np
from contextlib import ExitStack

S, B, D = 2048, 2, 1024
H, HD = 16, 64
SCALE = 1.0 / (HD ** 0.5)
P = 128
N_CORES = 8
CORES_PER_B = 4
NH = H // CORES_PER_B          # heads per core = 4
HDL = NH * HD                  # local head width = 256
CSHIFT = 2.75                  # exp shift: keeps p' in fp8e4m3 normal range
JVW = 80                       # V2 j-block stride: DoubleRow needs step%16==0
HVW = 2 * JVW                  # per-head V2 stride = 160
NV = NH * HVW                  # V2 row width = 640
QC = 512                       # q-chunk per attention stripe

_cache = {}


def _build(reps=1):
    import concourse.bacc as bacc
    import concourse.mybir as mybir
    from concourse import tile

    nc = bacc.Bacc("TRN2", target_bir_lowering=False, debug=False,
                   num_devices=N_CORES)

    F32 = mybir.dt.float32
    BF16 = mybir.dt.bfloat16
    x = nc.dram_tensor("x", [D, S], BF16, kind="ExternalInput")
    wkqv = nc.dram_tensor("wkqv", [D, 3 * HDL], BF16, kind="ExternalInput")
    bq = nc.dram_tensor("bq", [P, 2], F32, kind="ExternalInput")
    wp = nc.dram_tensor("wp", [HDL, D], BF16, kind="ExternalInput")
    y = nc.dram_tensor("y", [S, D], BF16, kind="ExternalOutput")

    with tile.TileContext(nc) as tc:
        k = _Kernel(nc, tc, mybir, x, wkqv, bq, wp, y)
        with ExitStack() as ctx:
            k.alloc(ctx)
            k.prologue()
            if reps == 1:
                fin = k.body(0)
                fin()
                k.flush(0)
            else:
                assert reps % 2 == 0
                with tc.For_i(0, reps // 2):
                    finA = k.body(0)
                    finB = k.body(1, prev_finale=finA)
                    finB()
                k.flush(1)
    nc.compile()
    return nc


class _Kernel:
    def __init__(self, nc, tc, mybir, x, wkqv, bq, wp, y):
        self.nc, self.tc, self.mybir = nc, tc, mybir
        self.x, self.wkqv, self.bq, self.wp, self.y = x, wkqv, bq, wp, y
        self.n_d = D // P            # 8
        self.n_t = S // P            # 16
        self.n_qc = S // QC          # 4
        self.n_r = self.n_t // 2     # 8

    # ---------------- allocation ----------------
    def alloc(self, ctx):
        nc, tc, mybir = self.nc, self.tc, self.mybir
        F32, BF16 = mybir.dt.float32, mybir.dt.bfloat16
        P_DT = mybir.dt.float8e4
        const = ctx.enter_context(tc.tile_pool(name="const", bufs=1))
        t = const.tile
        self.xb = [[t([P, S], BF16, tag=f"x{s}_{d}", name=f"x{s}_{d}")
                    for d in range(self.n_d)] for s in range(2)]
        self.wkqv_sb = [[t([P, 3 * HDL], BF16, tag=f"wkqv{s}_{d}",
                           name=f"wkqv{s}_{d}") for d in range(self.n_d)]
                        for s in range(2)]
        self.wk_sb = [[w[:, 0:HDL] for w in ws] for ws in self.wkqv_sb]
        self.wq_sb = [[w[:, HDL:2 * HDL] for w in ws] for ws in self.wkqv_sb]
        self.wv_sb = [[w[:, 2 * HDL:3 * HDL] for w in ws] for ws in self.wkqv_sb]
        self.bq_sb = t([P, 2], F32, tag="bq", name="bq")
        self.wp_sb = [[t([P, D], BF16, tag=f"wp{s}_{m}", name=f"wp{s}_{m}")
                       for m in range(2)] for s in range(2)]
        self.kT = [[t([P, S], BF16, tag=f"kT{s}_{m}", name=f"kT{s}_{m}")
                    for m in range(2)] for s in range(2)]
        self.qT = [[t([P, S], BF16, tag=f"qT{s}_{m}", name=f"qT{s}_{m}")
                    for m in range(2)] for s in range(2)]
        self.V2 = [[t([P, NV], P_DT, tag=f"V2{s}_{r}", name=f"V2{s}_{r}")
                    for r in range(self.n_r)] for s in range(2)]
        self.OT = [[t([P, S], BF16, tag=f"OT{s}_{m}", name=f"OT{s}_{m}")
                    for m in range(2)] for s in range(2)]
        self.csh = t([P, 1], F32, tag="csh", name="csh")
        self.work = ctx.enter_context(tc.tile_pool(name="work", bufs=1))
        self.ystream = ctx.enter_context(tc.tile_pool(name="ystream", bufs=4))
        self.rc_pool = ctx.enter_context(tc.tile_pool(name="rc", bufs=1))
        # PSUM: sc 2x2 banks + pv 2 + chain 2 = 8
        self.sc_pool = ctx.enter_context(
            tc.tile_pool(name="sc", bufs=1, space="PSUM"))
        self.pv_pool = ctx.enter_context(
            tc.tile_pool(name="pv", bufs=1, space="PSUM"))
        self.chain = ctx.enter_context(
            tc.tile_pool(name="chain", bufs=1, space="PSUM"))

    # ---------------- DMA ----------------
    def dma_in(self, st):
        """Issue input DMAs filling buffer set `st`.  Order matters: the
        first chains of the consuming body need wkqv + x[:, 0:512]."""
        nc = self.nc
        for d in range(self.n_d):
            nc.sync.dma_start(self.wkqv_sb[st][d][:],
                              self.wkqv[d * P:(d + 1) * P, :])
        for c in range(4):
            for d in range(self.n_d):
                nc.sync.dma_start(self.xb[st][d][:, c * 512:(c + 1) * 512],
                                  self.x[d * P:(d + 1) * P, c * 512:(c + 1) * 512])
        for m in range(2):
            nc.sync.dma_start(self.wp_sb[st][m][:],
                              self.wp[m * P:(m + 1) * P, :])

    # ---------------- chain producers (write set st) ----------------
    def kq_chunk(self, st, dst, wsb, m, lo, is_q):
        """Emitted in two ~860ns halves (d 0-3, d 4-7) so a single filler
        slot never delays the next score matmuls by more than ~1 slot."""
        nc, mybir = self.nc, self.mybir
        ps = self.chain.tile([P, 512], mybir.dt.float32, tag="kq", name="kq",
                             bufs=2)
        for d in range(self.n_d // 2):
            nc.tensor.matmul(ps[:], wsb[st][d][:, m * P:(m + 1) * P],
                             self.xb[st][d][:, lo:lo + 512],
                             start=(d == 0), stop=False)
        yield
        for d in range(self.n_d // 2, self.n_d):
            nc.tensor.matmul(ps[:], wsb[st][d][:, m * P:(m + 1) * P],
                             self.xb[st][d][:, lo:lo + 512],
                             start=False, stop=(d == self.n_d - 1))
        if is_q:
            nc.vector.tensor_scalar(dst[st][m][:, lo:lo + 512], ps[:],
                                    self.bq_sb[:, m:m + 1], None,
                                    op0=mybir.AluOpType.add)
        else:
            nc.vector.tensor_copy(dst[st][m][:, lo:lo + 512], ps[:])
        yield

    def v_tile(self, st, tt):
        """V~ for t-tile tt -> folded slot j=tt%2 of pair tile V2[st][tt//2]."""
        nc, mybir = self.nc, self.mybir
        vp = self.chain.tile([P, 512], mybir.dt.float32, tag="kq", name="vp",
                             bufs=2)
        for d in range(self.n_d):
            nc.tensor.matmul(vp[:, 0:HDL], self.xb[st][d][:, tt * P:(tt + 1) * P],
                             self.wv_sb[st][d][:],
                             start=(d == 0), stop=(d == self.n_d - 1))
        j = tt % 2
        dst = (self.V2[st][tt // 2][:, :]
               .rearrange("p (h c) -> p h c", h=NH)[:, :, j * JVW:j * JVW + 64])
        nc.vector.tensor_copy(
            dst, vp[:, 0:HDL].rearrange("p (h c) -> p h c", h=NH))

    # ---------------- projection (reads OT[st], wp[st]) ----------------
    def proj_steps(self, st, qc):
        """Projection of q-chunk qc: one 512-wide output block per step.
        8 steps total (4 qt-tiles x 2 n-blocks); the output DMA of each
        qt-tile is emitted before the second yield so 8 next() calls emit
        everything."""
        nc, mybir = self.nc, self.mybir
        qlo = qc * QC
        for qt in range(qlo // P, (qlo + QC) // P):
            yt = self.ystream.tile([P, D], mybir.dt.bfloat16, tag="yt",
                                   name="yt")
            for nn in range(0, D, 512):
                ps = self.chain.tile([P, 512], mybir.dt.float32, tag="kq",
                                     name="proj", bufs=2)
                for m in range(2):
                    nc.tensor.matmul(ps[:], self.OT[st][m][:, qt * P:(qt + 1) * P],
                                     self.wp_sb[st][m][:, nn:nn + 512],
                                     start=(m == 0), stop=(m == 1))
                nc.vector.tensor_copy(yt[:, nn:nn + 512], ps[:])
                if nn == 0:
                    yield
            nc.sync.dma_start(self.y[qt * P:(qt + 1) * P, :], yt[:])
            yield

    # ---------------- prologue ----------------
    def prologue(self):
        nc, mybir = self.nc, self.mybir
        self.dma_in(0)
        self.dma_in(1)
        nc.sync.dma_start(self.bq_sb[:], self.bq[:, :])
        # ones columns of V2 (softmax denominator rows), written once: the
        # v_tile copies only touch [:, :64] of each 80-wide j-block.
        for st in range(2):
            for r in range(self.n_r):
                col = self.V2[st][r][:, :].rearrange("p (h c) -> p h c", h=NH)
                for j in range(2):
                    nc.vector.memset(col[:, :, j * JVW + 64:j * JVW + 65], 1.0)
            for m in range(2):
                nc.vector.memset(self.OT[st][m][:], 0.0)
        nc.gpsimd.memset(self.csh[:], CSHIFT)
        # iteration-0 chain set
        for m in range(2):
            for lo in range(0, S, 512):
                for g in (self.kq_chunk(0, self.kT, self.wk_sb, m, lo, False),
                          self.kq_chunk(0, self.qT, self.wq_sb, m, lo, True)):
                    for _ in g:
                        pass
        for tt in range(self.n_t):
            self.v_tile(0, tt)

    # ---------------- filler schedule ----------------
    def filler(self, st, carry):
        """64 filler units (~860ns of PE work each), one per r-slot.
        st = set under production (the NEXT iteration's buffers); carry =
        leftover proj steps of the previous body.  The xb[st] data was
        DMA'd during the PREVIOUS body, so chain units can run from
        stripe 0.  Reserved slots: stripe 0 r0-3 = carry, stripe 5/6/7
        r0-3 = own proj qc0/1/2 (available after stripe 4+qc)."""
        nxt = st

        def gen_units():
            # 16 kq chunks (2 units each) + 16 v tiles (1 unit), woven 2:1
            chunks = ([(self.kT, self.wk_sb, m, lo, False)
                       for m in range(2) for lo in range(0, S, 512)]
                      + [(self.qT, self.wq_sb, m, lo, True)
                         for m in range(2) for lo in range(0, S, 512)])
            vts = list(range(self.n_t))
            ui = []
            for i, (dst, w, m, lo, is_q) in enumerate(chunks):
                g = self.kq_chunk(nxt, dst, w, m, lo, is_q)
                ui.append(lambda g=g: next(g, None))
                ui.append(lambda g=g: next(g, None))
                if vts:
                    ui.append(self._v_thunk(nxt, vts.pop(0)))
        # 16*3 = 48 units
            return ui

        def Pj(gen):
            return lambda: next(gen, None)

        own = [self.proj_steps(1 - nxt, qc) for qc in range(self.n_qc)]
        units = gen_units()
        sched = [[[] for _ in range(8)] for _ in range(8)]
        # reserved proj slots (2 proj steps per slot = ~850ns)
        for r in range(4):
            if carry is not None:
                sched[0][r] = [Pj(carry), Pj(carry)]
            for qc in range(3):
                sched[5 + qc][r] = [Pj(own[qc]), Pj(own[qc])]
        # fill remaining slots with chain/v units in order
        it = iter(units)
        for s in range(8):
            for r in range(8):
                if not sched[s][r]:
                    u = next(it, None)
                    if u is not None:
                        sched[s][r] = [u]
        rest = list(it)
        if rest:
            sched[7][7].extend(rest)
        # dma issue for the set consumed two bodies later (0 PE cost)
        sched[5][4].append(self._dma_thunk(1 - nxt))
        return sched

    def _v_thunk(self, st, tt):
        return lambda: self.v_tile(st, tt)

    def _dma_thunk(self, st):
        return lambda: self.dma_in(st)

    # ---------------- attention stripe ----------------
    def attention_pair(self, st, m, qc, slots, prev_finale=None):
        nc, mybir = self.nc, self.mybir
        AF = self.mybir.ActivationFunctionType
        F32 = mybir.dt.float32
        P_DT = mybir.dt.float8e4
        n_r = self.n_r
        qlo = qc * QC
        kT, qT, V2, OT = (self.kT[st], self.qT[st], self.V2[st], self.OT[st])
        pvs = [self.pv_pool.tile([65, QC], F32, tag=f"pv{half}",
                                 name=f"pv{half}") for half in range(2)]
        pTs = [[None] * n_r for _ in range(2)]

        def pv_mm(half, r):
            h = 2 * m + half
            lhsT = (V2[r][:, h * HVW:(h + 1) * HVW]
                    .rearrange("p (j c) -> p j c", j=2)[:, :, 0:65])
            rhs = pTs[half][r][:, :].rearrange("p (j n) -> p j n", j=2)
            nc.tensor.matmul(pvs[half][:], lhsT, rhs,
                             perf_mode=mybir.MatmulPerfMode.DoubleRow,
                             start=(r == 0), stop=(r == n_r - 1))

        for r in range(n_r):
            for half in range(2):
                plo = half * 64
                sc = self.sc_pool.tile([P, 2 * QC], F32, tag="sc", name="sc",
                                       bufs=2)
                for j in range(2):
                    nc.tensor.matmul(
                        sc[:, j * QC:(j + 1) * QC],
                        kT[m][plo:plo + 64, (2 * r + j) * P:(2 * r + j + 1) * P],
                        qT[m][plo:plo + 64, qlo:qlo + QC],
                        start=True, stop=True)
                pTs[half][r] = self.work.tile([P, 2 * QC], P_DT, tag="pT",
                                              name="pT", bufs=6)
                nc.scalar.activation(pTs[half][r][:], sc[:], AF.Exp,
                                     bias=self.csh[:, 0:1], scale=SCALE)
            if r == 0 and prev_finale is not None:
                # previous stripe's last pv pair + softmax drain, deferred
                # past this stripe's first scores so the ACT stream never
                # waits on the stripe turnaround.
                prev_finale()
            thunks = list(slots[r])
            if r > 0:
                pv_mm(0, r - 1)
                # one filler unit between the two pv matmuls: pv(h1) gates
                # on ACT(r-1,h1) which retires ~1.1us after ACT(r-1,h0).
                if thunks:
                    thunks.pop(0)()
                pv_mm(1, r - 1)
            for thunk in thunks:
                thunk()

        def finale():
            pv_mm(0, n_r - 1)
            pv_mm(1, n_r - 1)
            for half in range(2):
                plo = half * 64
                pv = pvs[half]
                # drain (releases pv).  reciprocal_approx_fast and partition
                # broadcasts mishandle nonzero base partitions on hardware,
                # so every DVE/Pool op below runs at base partition 0 and the
                # final multiply slices matching partition ranges.
                den = self.rc_pool.tile([1, QC], F32, tag="den", name="den",
                                        bufs=4)
                nc.vector.tensor_copy(den[:], pv[64:65, :])
                nc.vector.tensor_copy(OT[m][plo:plo + 64, qlo:qlo + QC],
                                      pv[0:64, :])
                rc1 = self.rc_pool.tile([1, QC], F32, tag="rc1", name="rc1",
                                        bufs=4)
                nc.vector.reciprocal_approx_fast(rc1[:], den[:])
                rcb = self.rc_pool.tile([P, QC], F32, tag="rcb", name="rcb",
                                        bufs=2)
                nc.gpsimd.partition_broadcast(rcb[:], rc1[0:1, :])
                nc.vector.tensor_tensor(OT[m][plo:plo + 64, qlo:qlo + QC],
                                        OT[m][plo:plo + 64, qlo:qlo + QC],
                                        rcb[plo:plo + 64, :],
                                        op=self.mybir.AluOpType.mult)

        return finale

    # ---------------- body ----------------
    def body(self, cur, prev_finale=None):
        nxt = 1 - cur
        # carry: previous body's proj(qc3) on the OTHER set.  On the first
        # pass this reads zeroed OT (prologue memset) and writes a harmless
        # zero y[qc3] block that later iterations overwrite; the flush
        # after the loop emits the final correct one.
        carry = self.proj_steps(1 - cur, 3)
        sched = self.filler(nxt, carry)
        fin = prev_finale
        si = 0
        for m in range(2):
            for qc in range(self.n_qc):
                fin = self.attention_pair(cur, m, qc, sched[si],
                                          prev_finale=fin)
                si += 1
        return fin

    def flush(self, last_cur):
        gen = self.proj_steps(last_cur, 3)
        for _ in range(8):
            next(gen, None)


def _get_nc(reps=1):
    if reps not in _cache:
        _cache[reps] = _build(reps=reps)
    return _cache[reps]


def make_in_maps(inputs, Wkv, bkv, Wq, bq, Wp, bp):
    """Host-side sharding: per-core input dicts (bf16)."""
    import ml_dtypes
    BF = ml_dtypes.bfloat16
    inputs = np.asarray(inputs, dtype=np.float32)
    Wkv = np.asarray(Wkv, dtype=np.float32)
    Wq = np.asarray(Wq, dtype=np.float32)
    bq = np.asarray(bq, dtype=np.float32)
    Wp = np.asarray(Wp, dtype=np.float32)

    in_maps = []
    for c in range(N_CORES):
        b = c // CORES_PER_B
        g = c % CORES_PER_B
        hsl = slice(g * HDL, (g + 1) * HDL)
        wkqv = np.concatenate([
            Wkv[:, hsl], Wq[:, hsl],
            Wkv[:, H * HD + g * HDL: H * HD + (g + 1) * HDL]], axis=1)
        in_maps.append(dict(
            x=np.ascontiguousarray(inputs[:, b, :].T).astype(BF),
            wkqv=np.ascontiguousarray(wkqv).astype(BF),
            bq=np.ascontiguousarray(bq[hsl].reshape(2, P).T),
            wp=np.ascontiguousarray(Wp[hsl, :]).astype(BF)))
    return in_maps


def combine_outputs(results):
    """Host-side unshard: sum the head-group partials per batch."""
    out = np.zeros((S, B, D), np.float32)
    for b in range(B):
        acc = results[b * CORES_PER_B]["y"].astype(np.float32)
        for g in range(1, CORES_PER_B):
            acc += results[b * CORES_PER_B + g]["y"].astype(np.float32)
        out[:, b, :] = acc
    return out


def kernel(inputs, Wkv, bkv, Wq, bq, Wp, bp):
    from concourse.bass_utils import run_bass_kernel_spmd
    nc = _get_nc()
    in_maps = make_in_maps(inputs, Wkv, bkv, Wq, bq, Wp, bp)
    res = run_bass_kernel_spmd(nc, in_maps, list(range(N_CORES)))
    out = combine_outputs(res.results)
    # bias terms hoisted off-device: y += bv @ Wp + bp  (softmax weights sum
    # to one, so the v-bias contributes a constant row through Wp)
    bkv64 = np.asarray(bkv, np.float64)
    bias = (bkv64[H * HD:] @ np.asarray(Wp, np.float64)
            + np.asarray(bp, np.float64)).astype(np.float32)
    out += bias[None, None, :]
    return out


# revision 14
# speedup vs baseline: 1.0885x; 1.0885x over previous
"""Trainium2 Bass kernel for nn_Attention_90125593739547.

Full-input contract: kernel(**inputs) takes the unsharded numpy inputs and
returns the full [S, B, D] output. Internally:
  - 8 NeuronCores, core c handles batch b = c // 4 and 4 heads (c % 4).
  - Softmax algebra moves biases off the TensorE: the k-bias shifts all
    logits of a softmax row equally (dropped), the v-bias and output bias
    are linear post-terms (added on host), only the q-bias survives (one
    per-partition DVE add at evacuation).

Scheduling (v2): the kernel is ScalarE-bound (128 exp ACTIVATEs of
(1024+352)/1.2 ns = 147us); the PE must stay >90% busy so the HAM clock
gate never drops it to 1.2 GHz.  The For_i body is unrolled 2x with
double-buffered K/Q/V sets: each body runs pure attention on the current
set while producing the NEXT iteration's K/Q/V + V2 tiles as evenly
spread PE filler.  A prologue outside the loop produces iteration 0's
set; each body's trailing projection (qc3) is carried into the next
body's first stripe; a flush after the loop emits the last one.

Per-core program (bf16 matmuls, fp8e4m3 DoubleRow attn@V):
  kT/qT = W.T @ x          [128 (2 heads x 64), 2048] bf16, N=2048 chains
  V2    = x @ Wv stored per t-tile-pair in a DoubleRow-folded layout
          [128, 4h * 2j * 68]; column 64 of each 68-block is memset to 1
          so the PV matmul also accumulates the softmax denominator.
  per head-pair (m), per q-chunk of 512:
    sc  = kT_h.T @ qT_h per t-pair          [128, 2 * 512] PSUM
    pT  = exp(SCALE * sc + C)               one ScalarE op; C keeps the
                                            fp8 values in normal range
                                            and cancels in the ratio
    pv += V2_pair.T @ pT  (DoubleRow K=256) [65, 512]; row 64 = sum p
    OT  = pv[0:64] * recip(pv[64])
  y_partial = OT.T @ Wp                     [2048, 1024] bf16 out
Host sums the 4 per-head-group partials per batch and adds bv@Wp + bp.
"""
import sys
sys.path.insert(0, '/opt/trn_rl_repo')
import numpy as np
from contextlib import ExitStack

S, B, D = 2048, 2, 1024
H, HD = 16, 64
SCALE = 1.0 / (HD ** 0.5)
P = 128
N_CORES = 8
CORES_PER_B = 4
NH = H // CORES_PER_B          # heads per core = 4
HDL = NH * HD                  # local head width = 256
CSHIFT = 2.75                  # exp shift: keeps p' in fp8e4m3 normal range
JVW = 80                       # V2 j-block stride: DoubleRow needs step%16==0
HVW = 2 * JVW                  # per-head V2 stride = 160
NV = NH * HVW                  # V2 row width = 640
QC = 512                       # q-chunk per attention stripe

_cache = {}


def _build(reps=1):
    import concourse.bacc as bacc
    import concourse.mybir as mybir
    from concourse import tile

    nc = bacc.Bacc("TRN2", target_bir_lowering=False, debug=False,
                   num_devices=N_CORES)

    F32 = mybir.dt.float32
    BF16 = mybir.dt.bfloat16
    x = nc.dram_tensor("x", [D, S], BF16, kind="ExternalInput")
    wkqv = nc.dram_tensor("wkqv", [D, 3 * HDL], BF16, kind="ExternalInput")
    bq = nc.dram_tensor("bq", [P, 2], F32, kind="ExternalInput")
    wp = nc.dram_tensor("wp", [HDL, D], BF16, kind="ExternalInput")
    y = nc.dram_tensor("y", [S, D], BF16, kind="ExternalOutput")

    with tile.TileContext(nc) as tc:
        k = _Kernel(nc, tc, mybir, x, wkqv, bq, wp, y)
        with ExitStack() as ctx:
            k.alloc(ctx)
            k.prologue()
            if reps == 1:
                fin = k.body(0)
                fin()
                k.flush(0)
            else:
                assert reps % 4 == 0
                with tc.For_i(0, reps // 4):
                    fin = None
                    for i in range(4):
                        fin = k.body(i % 2, prev_finale=fin)
                    fin()
                k.flush(1)
    nc.compile()
    return nc


class _Kernel:
    def __init__(self, nc, tc, mybir, x, wkqv, bq, wp, y):
        self.nc, self.tc, self.mybir = nc, tc, mybir
        self.x, self.wkqv, self.bq, self.wp, self.y = x, wkqv, bq, wp, y
        self.n_d = D // P            # 8
        self.n_t = S // P            # 16
        self.n_qc = S // QC          # 4
        self.n_r = self.n_t // 2     # 8

    # ---------------- allocation ----------------
    def alloc(self, ctx):
        nc, tc, mybir = self.nc, self.tc, self.mybir
        F32, BF16 = mybir.dt.float32, mybir.dt.bfloat16
        P_DT = mybir.dt.float8e4
        const = ctx.enter_context(tc.tile_pool(name="const", bufs=1))
        t = const.tile
        self.xb = [[t([P, S], BF16, tag=f"x{s}_{d}", name=f"x{s}_{d}")
                    for d in range(self.n_d)] for s in range(2)]
        self.wkqv_sb = [[t([P, 3 * HDL], BF16, tag=f"wkqv{s}_{d}",
                           name=f"wkqv{s}_{d}") for d in range(self.n_d)]
                        for s in range(2)]
        self.wk_sb = [[w[:, 0:HDL] for w in ws] for ws in self.wkqv_sb]
        self.wq_sb = [[w[:, HDL:2 * HDL] for w in ws] for ws in self.wkqv_sb]
        self.wv_sb = [[w[:, 2 * HDL:3 * HDL] for w in ws] for ws in self.wkqv_sb]
        self.bq_sb = t([P, 2], F32, tag="bq", name="bq")
        self.wp_sb = [[t([P, D], BF16, tag=f"wp{s}_{m}", name=f"wp{s}_{m}")
                       for m in range(2)] for s in range(2)]
        self.kT = [[t([P, S], BF16, tag=f"kT{s}_{m}", name=f"kT{s}_{m}")
                    for m in range(2)] for s in range(2)]
        self.qT = [[t([P, S], BF16, tag=f"qT{s}_{m}", name=f"qT{s}_{m}")
                    for m in range(2)] for s in range(2)]
        self.V2 = [[t([P, NV], P_DT, tag=f"V2{s}_{r}", name=f"V2{s}_{r}")
                    for r in range(self.n_r)] for s in range(2)]
        self.OT = [[t([P, S], BF16, tag=f"OT{s}_{m}", name=f"OT{s}_{m}")
                    for m in range(2)] for s in range(2)]
        self.csh = t([P, 1], F32, tag="csh", name="csh")
        self.work = ctx.enter_context(tc.tile_pool(name="work", bufs=1))
        self.ystream = ctx.enter_context(tc.tile_pool(name="ystream", bufs=4))
        self.rc_pool = ctx.enter_context(tc.tile_pool(name="rc", bufs=1))
        # PSUM: sc 2x2 banks + pv 2 + chain 2 = 8
        self.sc_pool = ctx.enter_context(
            tc.tile_pool(name="sc", bufs=1, space="PSUM"))
        self.pv_pool = ctx.enter_context(
            tc.tile_pool(name="pv", bufs=1, space="PSUM"))
        self.chain = ctx.enter_context(
            tc.tile_pool(name="chain", bufs=1, space="PSUM"))

    # ---------------- DMA ----------------
    def dma_in(self, st):
        """Issue input DMAs filling buffer set `st`.  Order matters: the
        first chains of the consuming body need wkqv + x[:, 0:512]."""
        nc = self.nc
        for d in range(self.n_d):
            nc.sync.dma_start(self.wkqv_sb[st][d][:],
                              self.wkqv[d * P:(d + 1) * P, :])
        for c in range(4):
            for d in range(self.n_d):
                nc.sync.dma_start(self.xb[st][d][:, c * 512:(c + 1) * 512],
                                  self.x[d * P:(d + 1) * P, c * 512:(c + 1) * 512])
        for m in range(2):
            nc.sync.dma_start(self.wp_sb[st][m][:],
                              self.wp[m * P:(m + 1) * P, :])

    # ---------------- chain producers (write set st) ----------------
    def kq_chunk(self, st, dst, wsb, m, lo, is_q):
        """Emitted in two ~860ns halves (d 0-3, d 4-7) so a single filler
        slot never delays the next score matmuls by more than ~1 slot."""
        nc, mybir = self.nc, self.mybir
        ps = self.chain.tile([P, 512], mybir.dt.float32, tag="kq", name="kq",
                             bufs=2)
        for d in range(self.n_d // 2):
            nc.tensor.matmul(ps[:], wsb[st][d][:, m * P:(m + 1) * P],
                             self.xb[st][d][:, lo:lo + 512],
                             start=(d == 0), stop=False)
        yield
        for d in range(self.n_d // 2, self.n_d):
            nc.tensor.matmul(ps[:], wsb[st][d][:, m * P:(m + 1) * P],
                             self.xb[st][d][:, lo:lo + 512],
                             start=False, stop=(d == self.n_d - 1))
        if is_q:
            nc.vector.tensor_scalar(dst[st][m][:, lo:lo + 512], ps[:],
                                    self.bq_sb[:, m:m + 1], None,
                                    op0=mybir.AluOpType.add)
        else:
            nc.vector.tensor_copy(dst[st][m][:, lo:lo + 512], ps[:])
        yield

    def v_tile(self, st, tt):
        """V~ for t-tile tt -> folded slot j=tt%2 of pair tile V2[st][tt//2]."""
        nc, mybir = self.nc, self.mybir
        vp = self.chain.tile([P, 512], mybir.dt.float32, tag="kq", name="vp",
                             bufs=2)
        for d in range(self.n_d):
            nc.tensor.matmul(vp[:, 0:HDL], self.xb[st][d][:, tt * P:(tt + 1) * P],
                             self.wv_sb[st][d][:],
                             start=(d == 0), stop=(d == self.n_d - 1))
        j = tt % 2
        dst = (self.V2[st][tt // 2][:, :]
               .rearrange("p (h c) -> p h c", h=NH)[:, :, j * JVW:j * JVW + 64])
        nc.vector.tensor_copy(
            dst, vp[:, 0:HDL].rearrange("p (h c) -> p h c", h=NH))

    # ---------------- projection (reads OT[st], wp[st]) ----------------
    def proj_steps(self, st, qc):
        """Projection of q-chunk qc: one 512-wide output block per step.
        8 steps total (4 qt-tiles x 2 n-blocks); the output DMA of each
        qt-tile is emitted before the second yield so 8 next() calls emit
        everything."""
        nc, mybir = self.nc, self.mybir
        qlo = qc * QC
        for qt in range(qlo // P, (qlo + QC) // P):
            yt = self.ystream.tile([P, D], mybir.dt.bfloat16, tag="yt",
                                   name="yt")
            for nn in range(0, D, 512):
                ps = self.chain.tile([P, 512], mybir.dt.float32, tag="kq",
                                     name="proj", bufs=2)
                for m in range(2):
                    nc.tensor.matmul(ps[:], self.OT[st][m][:, qt * P:(qt + 1) * P],
                                     self.wp_sb[st][m][:, nn:nn + 512],
                                     start=(m == 0), stop=(m == 1))
                nc.vector.tensor_copy(yt[:, nn:nn + 512], ps[:])
                if nn == 0:
                    yield
            nc.sync.dma_start(self.y[qt * P:(qt + 1) * P, :], yt[:])
            yield

    # ---------------- prologue ----------------
    def prologue(self):
        nc, mybir = self.nc, self.mybir
        self.dma_in(0)
        self.dma_in(1)
        nc.sync.dma_start(self.bq_sb[:], self.bq[:, :])
        # ones columns of V2 (softmax denominator rows), written once: the
        # v_tile copies only touch [:, :64] of each 80-wide j-block.
        for st in range(2):
            for r in range(self.n_r):
                col = self.V2[st][r][:, :].rearrange("p (h c) -> p h c", h=NH)
                for j in range(2):
                    nc.vector.memset(col[:, :, j * JVW + 64:j * JVW + 65], 1.0)
            for m in range(2):
                nc.vector.memset(self.OT[st][m][:], 0.0)
        nc.gpsimd.memset(self.csh[:], CSHIFT)
        # iteration-0 chain set
        for m in range(2):
            for lo in range(0, S, 512):
                for g in (self.kq_chunk(0, self.kT, self.wk_sb, m, lo, False),
                          self.kq_chunk(0, self.qT, self.wq_sb, m, lo, True)):
                    for _ in g:
                        pass
        for tt in range(self.n_t):
            self.v_tile(0, tt)

    # ---------------- filler schedule ----------------
    def filler(self, st, carry):
        """64 filler units (~860ns of PE work each), one per r-slot.
        st = set under production (the NEXT iteration's buffers); carry =
        leftover proj steps of the previous body.  The xb[st] data was
        DMA'd during the PREVIOUS body, so chain units can run from
        stripe 0.  Reserved slots: stripe 0 r0-3 = carry, stripe 5/6/7
        r0-3 = own proj qc0/1/2 (available after stripe 4+qc)."""
        nxt = st

        def gen_units():
            # 16 kq chunks (2 units each) + 16 v tiles (1 unit), woven 2:1
            chunks = ([(self.kT, self.wk_sb, m, lo, False)
                       for m in range(2) for lo in range(0, S, 512)]
                      + [(self.qT, self.wq_sb, m, lo, True)
                         for m in range(2) for lo in range(0, S, 512)])
            vts = list(range(self.n_t))
            ui = []
            for i, (dst, w, m, lo, is_q) in enumerate(chunks):
                g = self.kq_chunk(nxt, dst, w, m, lo, is_q)
                ui.append(lambda g=g: next(g, None))
                ui.append(lambda g=g: next(g, None))
                if vts:
                    ui.append(self._v_thunk(nxt, vts.pop(0)))
        # 16*3 = 48 units
            return ui

        def Pj(gen):
            return lambda: next(gen, None)

        own = [self.proj_steps(1 - nxt, qc) for qc in range(self.n_qc)]
        units = gen_units()
        sched = [[[] for _ in range(8)] for _ in range(8)]
        # reserved proj slots (2 proj steps per slot = ~850ns), at r4-7:
        # r0 runs the previous stripe's deferred finale whose DVE drain
        # chain (~2.3us) produces the OT these projections read — placing
        # them at r0-3 head-of-line-blocks the PE queue on that drain.
        for r in range(4, 8):
            if carry is not None:
                sched[0][r] = [Pj(carry), Pj(carry)]
            for qc in range(3):
                sched[5 + qc][r] = [Pj(own[qc]), Pj(own[qc])]
        # fill remaining slots with chain/v units in order
        it = iter(units)
        for s in range(8):
            for r in range(8):
                if not sched[s][r]:
                    u = next(it, None)
                    if u is not None:
                        sched[s][r] = [u]
        rest = list(it)
        if rest:
            sched[7][7].extend(rest)
        # dma issue for the set consumed two bodies later (0 PE cost)
        sched[5][4].append(self._dma_thunk(1 - nxt))
        return sched

    def _v_thunk(self, st, tt):
        return lambda: self.v_tile(st, tt)

    def _dma_thunk(self, st):
        return lambda: self.dma_in(st)

    # ---------------- attention stripe ----------------
    def attention_pair(self, st, m, qc, slots, prev_finale=None):
        nc, mybir = self.nc, self.mybir
        AF = self.mybir.ActivationFunctionType
        F32 = mybir.dt.float32
        P_DT = mybir.dt.float8e4
        n_r = self.n_r
        qlo = qc * QC
        kT, qT, V2, OT = (self.kT[st], self.qT[st], self.V2[st], self.OT[st])
        pvs = [self.pv_pool.tile([65, QC], F32, tag=f"pv{half}",
                                 name=f"pv{half}") for half in range(2)]
        pTs = [[None] * n_r for _ in range(2)]

        def pv_mm(half, r):
            h = 2 * m + half
            lhsT = (V2[r][:, h * HVW:(h + 1) * HVW]
                    .rearrange("p (j c) -> p j c", j=2)[:, :, 0:65])
            rhs = pTs[half][r][:, :].rearrange("p (j n) -> p j n", j=2)
            nc.tensor.matmul(pvs[half][:], lhsT, rhs,
                             perf_mode=mybir.MatmulPerfMode.DoubleRow,
                             start=(r == 0), stop=(r == n_r - 1))

        for r in range(n_r):
            for half in range(2):
                plo = half * 64
                sc = self.sc_pool.tile([P, 2 * QC], F32, tag="sc", name="sc",
                                       bufs=2)
                for j in range(2):
                    nc.tensor.matmul(
                        sc[:, j * QC:(j + 1) * QC],
                        kT[m][plo:plo + 64, (2 * r + j) * P:(2 * r + j + 1) * P],
                        qT[m][plo:plo + 64, qlo:qlo + QC],
                        start=True, stop=True)
                pTs[half][r] = self.work.tile([P, 2 * QC], P_DT, tag="pT",
                                              name="pT", bufs=6)
                nc.scalar.activation(pTs[half][r][:], sc[:], AF.Exp,
                                     bias=self.csh[:, 0:1], scale=SCALE)
            if r == 0 and prev_finale is not None:
                # previous stripe's last pv pair + softmax drain, deferred
                # past this stripe's first scores so the ACT stream never
                # waits on the stripe turnaround.
                prev_finale()
            thunks = list(slots[r])
            if r > 0:
                pv_mm(0, r - 1)
                # one filler unit between the two pv matmuls: pv(h1) gates
                # on ACT(r-1,h1) which retires ~1.1us after ACT(r-1,h0).
                if thunks:
                    thunks.pop(0)()
                pv_mm(1, r - 1)
            for thunk in thunks:
                thunk()

        def finale():
            pv_mm(0, n_r - 1)
            pv_mm(1, n_r - 1)
            for half in range(2):
                plo = half * 64
                pv = pvs[half]
                # drain (releases pv).  reciprocal_approx_fast and partition
                # broadcasts mishandle nonzero base partitions on hardware,
                # so every DVE/Pool op below runs at base partition 0 and the
                # final multiply slices matching partition ranges.
                den = self.rc_pool.tile([1, QC], F32, tag="den", name="den",
                                        bufs=4)
                nc.vector.tensor_copy(den[:], pv[64:65, :])
                nc.vector.tensor_copy(OT[m][plo:plo + 64, qlo:qlo + QC],
                                      pv[0:64, :])
                rc1 = self.rc_pool.tile([1, QC], F32, tag="rc1", name="rc1",
                                        bufs=4)
                nc.vector.reciprocal_approx_fast(rc1[:], den[:])
                rcb = self.rc_pool.tile([P, QC], F32, tag="rcb", name="rcb",
                                        bufs=2)
                nc.gpsimd.partition_broadcast(rcb[:], rc1[0:1, :])
                nc.vector.tensor_tensor(OT[m][plo:plo + 64, qlo:qlo + QC],
                                        OT[m][plo:plo + 64, qlo:qlo + QC],
                                        rcb[plo:plo + 64, :],
                                        op=self.mybir.AluOpType.mult)

        return finale

    # ---------------- body ----------------
    def body(self, cur, prev_finale=None):
        nxt = 1 - cur
        # carry: previous body's proj(qc3) on the OTHER set.  On the first
        # pass this reads zeroed OT (prologue memset) and writes a harmless
        # zero y[qc3] block that later iterations overwrite; the flush
        # after the loop emits the final correct one.
        carry = self.proj_steps(1 - cur, 3)
        sched = self.filler(nxt, carry)
        fin = prev_finale
        si = 0
        for m in range(2):
            for qc in range(self.n_qc):
                fin = self.attention_pair(cur, m, qc, sched[si],
                                          prev_finale=fin)
                si += 1
        return fin

    def flush(self, last_cur):
        gen = self.proj_steps(last_cur, 3)
        for _ in range(8):
            next(gen, None)


def _get_nc(reps=1):
    if reps not in _cache:
        _cache[reps] = _build(reps=reps)
    return _cache[reps]


def make_in_maps(inputs, Wkv, bkv, Wq, bq, Wp, bp):
    """Host-side sharding: per-core input dicts (bf16)."""
    import ml_dtypes
    BF = ml_dtypes.bfloat16
    inputs = np.asarray(inputs, dtype=np.float32)
    Wkv = np.asarray(Wkv, dtype=np.float32)
    Wq = np.asarray(Wq, dtype=np.float32)
    bq = np.asarray(bq, dtype=np.float32)
    Wp = np.asarray(Wp, dtype=np.float32)

    in_maps = []
    for c in range(N_CORES):
        b = c // CORES_PER_B
        g = c % CORES_PER_B
        hsl = slice(g * HDL, (g + 1) * HDL)
        wkqv = np.concatenate([
            Wkv[:, hsl], Wq[:, hsl],
            Wkv[:, H * HD + g * HDL: H * HD + (g + 1) * HDL]], axis=1)
        in_maps.append(dict(
            x=np.ascontiguousarray(inputs[:, b, :].T).astype(BF),
            wkqv=np.ascontiguousarray(wkqv).astype(BF),
            bq=np.ascontiguousarray(bq[hsl].reshape(2, P).T),
            wp=np.ascontiguousarray(Wp[hsl, :]).astype(BF)))
    return in_maps


def combine_outputs(results):
    """Host-side unshard: sum the head-group partials per batch."""
    out = np.zeros((S, B, D), np.float32)
    for b in range(B):
        acc = results[b * CORES_PER_B]["y"].astype(np.float32)
        for g in range(1, CORES_PER_B):
            acc += results[b * CORES_PER_B + g]["y"].astype(np.float32)
        out[:, b, :] = acc
    return out


def kernel(inputs, Wkv, bkv, Wq, bq, Wp, bp):
    from concourse.bass_utils import run_bass_kernel_spmd
    nc = _get_nc()
    in_maps = make_in_maps(inputs, Wkv, bkv, Wq, bq, Wp, bp)
    res = run_bass_kernel_spmd(nc, in_maps, list(range(N_CORES)))
    out = combine_outputs(res.results)
    # bias terms hoisted off-device: y += bv @ Wp + bp  (softmax weights sum
    # to one, so the v-bias contributes a constant row through Wp)
    bkv64 = np.asarray(bkv, np.float64)
    bias = (bkv64[H * HD:] @ np.asarray(Wp, np.float64)
            + np.asarray(bp, np.float64)).astype(np.float32)
    out += bias[None, None, :]
    return out
